# revision 19
# baseline (speedup 1.0000x reference)
"""LRU single-step kernel for 8x TRN2 NeuronCores (Bass/Tile).

Math (per batch row b, hidden h):
  out_re[b,h] = lam_re[h]*h_re[b,h] - lam_im[h]*h_im[b,h] + (x @ (scale*B_real).T)[b,h]
  out_im[b,h] = lam_im[h]*h_re[b,h] + lam_re[h]*h_im[b,h] + (x @ (scale*B_img ).T)[b,h]

Strategy: data-parallel over the batch axis (8 shards of 32768 rows), computed
in a transposed layout (hidden on partitions, batch on the free axis). The
kernel is memory-bound, so precision is chosen per-stream to minimize HBM
bytes inside the rel-err budget:
  - x and the projection weights travel as bf16 (fp8 x fails the error gate:
    the exp(gamma) scaling amplifies x's quantization error),
  - h_re / h_im travel as fp8 e4m3 (their error is attenuated by |Lambda|<1),
  - outputs travel as bf16.

Engine split per 512-column block (PSUM bank) and hidden chunk:
  PE:   ps_re = W_re.T @ x + diag(lam_re) @ hre      (bf16 x fp8-moving)
        ps_im = W_im.T @ x + diag(lam_im) @ hre
  DVE:  ore = (him * -lam_im[p]) + ps_re             (scalar_tensor_tensor,
        oim = (him *  lam_re[p]) + ps_im              fp32 scalar, bf16 out)
The him Lambda terms ride the PSUM->SBUF drain for free, cutting PE matmuls
from 6 to 4 per block pair. Only DVE/ACT can read PSUM (GPSIMD can't, and
ACT's activation has no tensor bias), so all drains live on DVE; to fit them
under the DMA roofline each drain covers FOUR adjacent PSUM banks (2048
columns, half of PSUM) in one instruction, amortizing the fixed access
latency. The two 4-bank quad tiles ping-pong: the PE fills one while the DVE
drains the other.

DMA queues: loads on the SP ring (nothing else queued there, so prefetch never
blocks behind a store waiting on compute), stores on the ACT ring.

PE Matmult instructions only have one sync-wait slot in codegen, so waits are
carefully absorbed before real matmuls run:
  - per-iteration 1x1 "lane absorber" matmuls read one freshly-DMA'd tile each
    (and write a persistent scratch PSUM tile), so each carries exactly one
    DMA-lane wait and advances the PE's observed clock;
  - PSUM tiles are allocated once and reused manually (no pool recycling), so
    no TileRelease edges exist on PSUM: the first matmul of a group carries
    only the WAR wait on the previous iteration's PSUM->SBUF drain.
"""

import numpy as np
import ml_dtypes

import concourse.bass as bass
import concourse.mybir as mybir
from concourse.tile import TileContext
from concourse.bass_utils import run_bass_kernel_spmd

B_SZ, IN_DIM, HID = 262144, 128, 256
N_CORES = 8
S = B_SZ // N_CORES  # 32768 rows per core
P = 128
HCHUNKS = HID // P  # 2
COLS = 4096          # batch columns per outer iteration (8 KiB bf16 lines)
OUTER = S // COLS    # 8
MMF = 512            # matmul free dim (one fp32 PSUM bank)
NBLK = COLS // MMF   # 8
QUAD = 2048          # DVE drain width (four PSUM banks)

# bf16 consts layout (one (128, 1024) tensor):
#   [:, 0:256]     w_re  = (scale*B_real).T
#   [:, 256:512]   w_im  = (scale*B_img).T
#   [:, 512:768]   diag(lam_re)  chunks 0,1
#   [:, 768:1024]  diag(lam_im)  chunks 0,1
CONST_COLS = 1024
# fp32 per-partition scalars (128, 4): [-lam_im c0, -lam_im c1, lam_re c0, lam_re c1]
LAMF_COLS = 4

BF16 = mybir.dt.bfloat16
FP8 = mybir.dt.float8e4
F32 = mybir.dt.float32
NP_BF16 = ml_dtypes.bfloat16
NP_FP8 = ml_dtypes.float8_e4m3fn

_cache = {}

# Stashed BassKernelResults from the most recent run (for test harnesses).
LAST_RESULTS = None


def _build():
    if "nc" in _cache:
        return _cache["nc"]

    nc = bass.Bass(trn_type="TRN2")

    x_t = nc.dram_tensor("x_t", (IN_DIM, S), BF16, kind="ExternalInput")
    hre_t = nc.dram_tensor("hre_t", (HID, S), FP8, kind="ExternalInput")
    him_t = nc.dram_tensor("him_t", (HID, S), FP8, kind="ExternalInput")
    consts = nc.dram_tensor("consts", (P, CONST_COLS), BF16, kind="ExternalInput")
    lamf = nc.dram_tensor("lamf", (P, LAMF_COLS), F32, kind="ExternalInput")

    o_re = nc.dram_tensor("o_re", (HID, S), BF16, kind="ExternalOutput")
    o_im = nc.dram_tensor("o_im", (HID, S), BF16, kind="ExternalOutput")

    hre_v = hre_t[:, :].rearrange("(c p) s -> p c s", p=P)
    him_v = him_t[:, :].rearrange("(c p) s -> p c s", p=P)
    ore_v = o_re[:, :].rearrange("(c p) s -> p c s", p=P)
    oim_v = o_im[:, :].rearrange("(c p) s -> p c s", p=P)

    with TileContext(nc) as tc:
        with (
            tc.tile_pool(name="cpool", bufs=1) as cpool,
            tc.tile_pool(name="xin", bufs=4) as xin,
            tc.tile_pool(name="hin", bufs=4) as hin,
            tc.tile_pool(name="outp", bufs=3) as outp,
            tc.tile_pool(name="psum", bufs=1, space="PSUM") as psum,
        ):
            csb = cpool.tile([P, CONST_COLS], BF16)
            lamsb = cpool.tile([P, LAMF_COLS], F32, tag="lamf")
            nc.sync.dma_start(csb[:], consts[:, :])
            nc.sync.dma_start(lamsb[:], lamf[:, :])
            # 2 persistent 4-bank PSUM quad tiles (all 16 KiB of PSUM);
            # allocated once so no TileRelease/realloc wait sets ever form on
            # PSUM. Each quad is bank-aligned (2048 f32 = exactly 4 banks):
            # matmuls write each 512 half within a single bank while the DVE
            # drain reads the whole 2048 columns in one go.
            ps_tiles = [psum.tile([P, QUAD], F32, tag=f"ps{i}", name=f"ps{i}")
                        for i in range(2)]
            _cache["ps_idx"] = 0

            def lane_absorb(tile_ap):
                # 1x1 matmul reading the freshly-DMA'd tile: carries exactly
                # one DMA-lane wait, advancing the PE's observed clock so the
                # real matmuls don't re-wait on that lane. The write lands in
                # the corner of the quad the next real group will overwrite
                # (start=True covers it), so no scratch bank is needed.
                quad = ps_tiles[_cache["ps_idx"] % 2]
                nc.tensor.matmul(quad[0:1, 0:1], tile_ap, tile_ap,
                                 start=True, stop=True, skip_group_check=True)

            w_re_sb = csb[:, 0:HID]
            w_im_sb = csb[:, HID:2 * HID]

            def dre_c(c):
                return csb[:, 2 * HID + c * P: 2 * HID + (c + 1) * P]

            def dim_c(c):
                return csb[:, 3 * HID + c * P: 3 * HID + (c + 1) * P]

            lane_absorb(csb[0:1, 0:1])

            for o in range(OUTER):
                sl = slice(o * COLS, (o + 1) * COLS)
                xt = xin.tile([P, COLS], BF16)
                nc.sync.dma_start(xt[:], x_t[:, sl])
                hre = hin.tile([P, HCHUNKS, COLS], FP8, tag="hre")
                him = hin.tile([P, HCHUNKS, COLS], FP8, tag="him")
                nc.sync.dma_start(hre[:], hre_v[:, :, sl])
                nc.sync.dma_start(him[:], him_v[:, :, sl])
                lane_absorb(xt[0:1, 0:1])
                lane_absorb(hre[0:1, 0, 0:1])
                lane_absorb(him[0:1, 0, 0:1])

                ore = outp.tile([P, HCHUNKS, COLS], BF16, tag="ore")
                oim = outp.tile([P, HCHUNKS, COLS], BF16, tag="oim")

                for c in range(HCHUNKS):
                    wre_c = w_re_sb[:, c * P:(c + 1) * P]
                    wim_c = w_im_sb[:, c * P:(c + 1) * P]
                    for qb in range(COLS // QUAD):
                        qs = slice(qb * QUAD, (qb + 1) * QUAD)
                        hims = him[:, c, qs]

                        quad_re = ps_tiles[_cache["ps_idx"] % 2]
                        _cache["ps_idx"] += 1
                        for k in range(QUAD // MMF):
                            hk = slice(qb * QUAD + k * MMF, qb * QUAD + (k + 1) * MMF)
                            bank = quad_re[:, k * MMF:(k + 1) * MMF]
                            nc.tensor.matmul(bank, wre_c, xt[:, hk],
                                             start=True, stop=False)
                            nc.tensor.matmul(bank, dre_c(c), hre[:, c, hk],
                                             start=False, stop=True)
                        nc.vector.scalar_tensor_tensor(
                            ore[:, c, qs], hims, lamsb[:, c:c + 1], quad_re[:],
                            op0=mybir.AluOpType.mult, op1=mybir.AluOpType.add)

                        quad_im = ps_tiles[_cache["ps_idx"] % 2]
                        _cache["ps_idx"] += 1
                        # The last im quad of the iteration drains on ACT
                        # instead (plain copy), with its him Lambda term as a
                        # third PE matmul: this pulls DVE below the DMA
                        # roofline and lets the two tail quads drain in
                        # parallel.
                        act_drain = (c == HCHUNKS - 1 and qb == COLS // QUAD - 1)
                        for k in range(QUAD // MMF):
                            hk = slice(qb * QUAD + k * MMF, qb * QUAD + (k + 1) * MMF)
                            bank = quad_im[:, k * MMF:(k + 1) * MMF]
                            nc.tensor.matmul(bank, wim_c, xt[:, hk],
                                             start=True, stop=False)
                            if act_drain:
                                nc.tensor.matmul(bank, dim_c(c), hre[:, c, hk],
                                                 start=False, stop=False)
                                nc.tensor.matmul(bank, dre_c(c), him[:, c, hk],
                                                 start=False, stop=True)
                            else:
                                nc.tensor.matmul(bank, dim_c(c), hre[:, c, hk],
                                                 start=False, stop=True)
                        if act_drain:
                            nc.scalar.copy(oim[:, c, qs], quad_im[:])
                        else:
                            nc.vector.scalar_tensor_tensor(
                                oim[:, c, qs], hims, lamsb[:, 2 + c:3 + c],
                                quad_im[:],
                                op0=mybir.AluOpType.mult, op1=mybir.AluOpType.add)

                    # Per-chunk stores on the ACT DGE ring (loads have the SP
                    # ring to themselves): chunk 0's outputs stream out while
                    # chunk 1 still computes, smoothing HBM write traffic and
                    # shortening the tail.
                    nc.scalar.dma_start(ore_v[:, c:c + 1, sl], ore[:, c:c + 1, :])
                    nc.scalar.dma_start(oim_v[:, c:c + 1, sl], oim[:, c:c + 1, :])

    _split_multiwaits(nc)
    _cache["nc"] = nc
    return nc


def _split_multiwaits(nc):
    """walrus codegen allows exactly one semaphore wait per instruction.
    Move all-but-one wait of every multi-wait instruction onto single-wait
    NOP instructions spliced immediately before it on the same engine
    (engines execute their stream in order, so semantics are unchanged)."""
    k = 0
    for bb in nc.m.functions[0].blocks:
        new_list = []
        for ins in bb.instructions:
            si = ins.sync_info
            if si is not None and si.on_wait and len(si.on_wait) > 1:
                for w in si.on_wait[:-1]:
                    nop = mybir.InstNoOp(
                        name=f"WN-{k}", engine=ins.engine,
                        sync_info=mybir.SyncInfo(on_wait=[w], on_update=[]),
                    )
                    k += 1
                    new_list.append(nop)
                si.on_wait = [si.on_wait[-1]]
            new_list.append(ins)
        bb.instructions[:] = new_list


def kernel(inputs, h_re, h_im, nu_log, theta_log, B_real, B_img, gamma_log):
    global LAST_RESULTS
    inputs = np.asarray(inputs, dtype=np.float32)
    h_re = np.asarray(h_re, dtype=np.float32)
    h_im = np.asarray(h_im, dtype=np.float32)
    nu_log = np.asarray(nu_log, dtype=np.float32)
    theta_log = np.asarray(theta_log, dtype=np.float32)
    B_real = np.asarray(B_real, dtype=np.float32)
    B_img = np.asarray(B_img, dtype=np.float32)
    gamma_log = np.asarray(gamma_log, dtype=np.float32)

    # Tiny parameter math on host (matches the f32 reference computation).
    mag = np.exp(-np.exp(nu_log))          # (1, H)
    theta = np.exp(theta_log)              # (1, H)
    lam_re = (mag * np.cos(theta))[0]      # (H,)
    lam_im = (mag * np.sin(theta))[0]      # (H,)
    scale = np.exp(gamma_log).T            # (H, 1)
    w_re = (scale * B_real).T              # (IN_DIM, H)
    w_im = (scale * B_img).T               # (IN_DIM, H)

    consts = np.zeros((P, CONST_COLS), np.float32)
    consts[:, 0:HID] = w_re
    consts[:, HID:2 * HID] = w_im
    idx = np.arange(P)
    lamf = np.zeros((P, LAMF_COLS), np.float32)
    for c in range(HCHUNKS):
        lr = lam_re[c * P:(c + 1) * P]
        li = lam_im[c * P:(c + 1) * P]
        consts[idx, 2 * HID + c * P + idx] = lr
        consts[idx, 3 * HID + c * P + idx] = li
        lamf[:, c] = -li
        lamf[:, 2 + c] = lr
    consts = consts.astype(NP_BF16)

    x_bf = inputs.astype(NP_BF16)
    hre_f8 = h_re.astype(NP_FP8)
    him_f8 = h_im.astype(NP_FP8)

    in_maps = []
    for core in range(N_CORES):
        sl = slice(core * S, (core + 1) * S)
        in_maps.append({
            "x_t": np.ascontiguousarray(x_bf[sl].T),
            "hre_t": np.ascontiguousarray(hre_f8[sl].T),
            "him_t": np.ascontiguousarray(him_f8[sl].T),
            "consts": consts,
            "lamf": lamf,
        })

    nc = _build()
    res = run_bass_kernel_spmd(nc, in_maps, core_ids=list(range(N_CORES)))
    LAST_RESULTS = res

    out = np.empty((2, B_SZ, HID), np.float32)
    for core in range(N_CORES):
        sl = slice(core * S, (core + 1) * S)
        out[0, sl] = res.results[core]["o_re"].T.astype(np.float32)
        out[1, sl] = res.results[core]["o_im"].T.astype(np.float32)
    return out


# revision 20
# speedup vs baseline: 1.0796x; 1.0796x over previous
"""LRU single-step kernel for 8x TRN2 NeuronCores (Bass/Tile).

Math (per batch row b, hidden h):
  out_re[b,h] = lam_re[h]*h_re[b,h] - lam_im[h]*h_im[b,h] + (x @ (scale*B_real).T)[b,h]
  out_im[b,h] = lam_im[h]*h_re[b,h] + lam_re[h]*h_im[b,h] + (x @ (scale*B_img ).T)[b,h]

Strategy: data-parallel over the batch axis (8 shards of 32768 rows), computed
in a transposed layout (hidden on partitions, batch on the free axis). The
kernel is memory-bound, so precision is chosen per-stream to minimize HBM
bytes inside the rel-err budget:
  - x and the projection weights travel as bf16 (fp8 x fails the error gate:
    the exp(gamma) scaling amplifies x's quantization error),
  - h_re / h_im travel as fp8 e4m3 (their error is attenuated by |Lambda|<1),
  - outputs travel as bf16.

Engine split per 512-column block (PSUM bank) and hidden chunk:
  PE:   ps_re = W_re.T @ x + diag(lam_re) @ hre      (bf16 x fp8-moving)
        ps_im = W_im.T @ x + diag(lam_im) @ hre
  DVE:  ore = (him * -lam_im[p]) + ps_re             (scalar_tensor_tensor,
        oim = (him *  lam_re[p]) + ps_im              fp32 scalar, bf16 out)
The him Lambda terms ride the PSUM->SBUF drain for free, cutting PE matmuls
from 6 to 4 per block pair. Only DVE/ACT can read PSUM (GPSIMD can't, and
ACT's activation has no tensor bias), so all drains live on DVE; to fit them
under the DMA roofline each drain covers FOUR adjacent PSUM banks (2048
columns, half of PSUM) in one instruction, amortizing the fixed access
latency. The two 4-bank quad tiles ping-pong: the PE fills one while the DVE
drains the other.

DMA queues: loads on the SP ring (nothing else queued there, so prefetch never
blocks behind a store waiting on compute), stores on the ACT ring.

PE Matmult instructions only have one sync-wait slot in codegen, so waits are
carefully absorbed before real matmuls run:
  - per-iteration 1x1 "lane absorber" matmuls read one freshly-DMA'd tile each
    (and write a persistent scratch PSUM tile), so each carries exactly one
    DMA-lane wait and advances the PE's observed clock;
  - PSUM tiles are allocated once and reused manually (no pool recycling), so
    no TileRelease edges exist on PSUM: the first matmul of a group carries
    only the WAR wait on the previous iteration's PSUM->SBUF drain.
"""

import numpy as np
import ml_dtypes

import concourse.bass as bass
import concourse.mybir as mybir
from concourse.tile import TileContext
from concourse.bass_utils import run_bass_kernel_spmd

B_SZ, IN_DIM, HID = 262144, 128, 256
N_CORES = 8
S = B_SZ // N_CORES  # 32768 rows per core
P = 128
HCHUNKS = HID // P  # 2
COLS = 4096          # batch columns per outer iteration (8 KiB bf16 lines)
OUTER = S // COLS    # 8
MMF = 512            # matmul free dim (one fp32 PSUM bank)
NBLK = COLS // MMF   # 8
QUAD = 2048          # DVE drain width (four PSUM banks)

# bf16 consts layout (one (128, 1024) tensor):
#   [:, 0:256]     w_re  = (scale*B_real).T
#   [:, 256:512]   w_im  = (scale*B_img).T
#   [:, 512:768]   diag(lam_re)  chunks 0,1
#   [:, 768:1024]  diag(lam_im)  chunks 0,1
CONST_COLS = 1024
# fp32 per-partition scalars (128, 4): [-lam_im c0, -lam_im c1, lam_re c0, lam_re c1]
LAMF_COLS = 4

BF16 = mybir.dt.bfloat16
FP8 = mybir.dt.float8e4
F32 = mybir.dt.float32
NP_BF16 = ml_dtypes.bfloat16
NP_FP8 = ml_dtypes.float8_e4m3fn

_cache = {}

# Stashed BassKernelResults from the most recent run (for test harnesses).
LAST_RESULTS = None


def _build():
    if "nc" in _cache:
        return _cache["nc"]

    nc = bass.Bass(trn_type="TRN2")

    x_t = nc.dram_tensor("x_t", (IN_DIM, S), BF16, kind="ExternalInput")
    hre_t = nc.dram_tensor("hre_t", (HID, S), FP8, kind="ExternalInput")
    him_t = nc.dram_tensor("him_t", (HID, S), FP8, kind="ExternalInput")
    consts = nc.dram_tensor("consts", (P, CONST_COLS), BF16, kind="ExternalInput")
    lamf = nc.dram_tensor("lamf", (P, LAMF_COLS), F32, kind="ExternalInput")

    o_re = nc.dram_tensor("o_re", (HID, S), BF16, kind="ExternalOutput")
    o_im = nc.dram_tensor("o_im", (HID, S), BF16, kind="ExternalOutput")

    hre_v = hre_t[:, :].rearrange("(c p) s -> p c s", p=P)
    him_v = him_t[:, :].rearrange("(c p) s -> p c s", p=P)
    ore_v = o_re[:, :].rearrange("(c p) s -> p c s", p=P)
    oim_v = o_im[:, :].rearrange("(c p) s -> p c s", p=P)

    with TileContext(nc) as tc:
        with (
            tc.tile_pool(name="cpool", bufs=1) as cpool,
            tc.tile_pool(name="xin", bufs=4) as xin,
            tc.tile_pool(name="hin", bufs=4) as hin,
            tc.tile_pool(name="outp", bufs=3) as outp,
            tc.tile_pool(name="psum", bufs=1, space="PSUM") as psum,
        ):
            csb = cpool.tile([P, CONST_COLS], BF16)
            lamsb = cpool.tile([P, LAMF_COLS], F32, tag="lamf")
            nc.sync.dma_start(csb[:], consts[:, :])
            nc.sync.dma_start(lamsb[:], lamf[:, :])
            # 2 persistent 4-bank PSUM quad tiles (all 16 KiB of PSUM);
            # allocated once so no TileRelease/realloc wait sets ever form on
            # PSUM. Each quad is bank-aligned (2048 f32 = exactly 4 banks):
            # matmuls write each 512 half within a single bank while the DVE
            # drain reads the whole 2048 columns in one go.
            ps_tiles = [psum.tile([P, QUAD], F32, tag=f"ps{i}", name=f"ps{i}")
                        for i in range(2)]
            _cache["ps_idx"] = 0

            def lane_absorb(tile_ap):
                # 1x1 matmul reading the freshly-DMA'd tile: carries exactly
                # one DMA-lane wait, advancing the PE's observed clock so the
                # real matmuls don't re-wait on that lane. The write lands in
                # the corner of the quad the next real group will overwrite
                # (start=True covers it), so no scratch bank is needed.
                quad = ps_tiles[_cache["ps_idx"] % 2]
                nc.tensor.matmul(quad[0:1, 0:1], tile_ap, tile_ap,
                                 start=True, stop=True, skip_group_check=True)

            w_re_sb = csb[:, 0:HID]
            w_im_sb = csb[:, HID:2 * HID]

            def dre_c(c):
                return csb[:, 2 * HID + c * P: 2 * HID + (c + 1) * P]

            def dim_c(c):
                return csb[:, 3 * HID + c * P: 3 * HID + (c + 1) * P]

            lane_absorb(csb[0:1, 0:1])

            for o in range(OUTER):
                sl = slice(o * COLS, (o + 1) * COLS)
                xt = xin.tile([P, COLS], BF16)
                nc.sync.dma_start(xt[:], x_t[:, sl])
                hre = hin.tile([P, HCHUNKS, COLS], FP8, tag="hre")
                him = hin.tile([P, HCHUNKS, COLS], FP8, tag="him")
                nc.sync.dma_start(hre[:], hre_v[:, :, sl])
                nc.sync.dma_start(him[:], him_v[:, :, sl])
                lane_absorb(xt[0:1, 0:1])
                lane_absorb(hre[0:1, 0, 0:1])
                lane_absorb(him[0:1, 0, 0:1])

                ore = outp.tile([P, HCHUNKS, COLS], BF16, tag="ore")
                oim = outp.tile([P, HCHUNKS, COLS], BF16, tag="oim")

                for c in range(HCHUNKS):
                    wre_c = w_re_sb[:, c * P:(c + 1) * P]
                    wim_c = w_im_sb[:, c * P:(c + 1) * P]
                    for qb in range(COLS // QUAD):
                        qs = slice(qb * QUAD, (qb + 1) * QUAD)
                        hims = him[:, c, qs]

                        quad_re = ps_tiles[_cache["ps_idx"] % 2]
                        _cache["ps_idx"] += 1
                        for k in range(QUAD // MMF):
                            hk = slice(qb * QUAD + k * MMF, qb * QUAD + (k + 1) * MMF)
                            bank = quad_re[:, k * MMF:(k + 1) * MMF]
                            nc.tensor.matmul(bank, wre_c, xt[:, hk],
                                             start=True, stop=False)
                            nc.tensor.matmul(bank, dre_c(c), hre[:, c, hk],
                                             start=False, stop=True)
                        nc.vector.scalar_tensor_tensor(
                            ore[:, c, qs], hims, lamsb[:, c:c + 1], quad_re[:],
                            op0=mybir.AluOpType.mult, op1=mybir.AluOpType.add)

                        quad_im = ps_tiles[_cache["ps_idx"] % 2]
                        _cache["ps_idx"] += 1
                        # The last im quad of the iteration drains on ACT
                        # instead (plain copy), with its him Lambda term as a
                        # third PE matmul: this pulls DVE below the DMA
                        # roofline and lets the two tail quads drain in
                        # parallel.
                        act_drain = (c == HCHUNKS - 1 and qb == COLS // QUAD - 1)
                        for k in range(QUAD // MMF):
                            hk = slice(qb * QUAD + k * MMF, qb * QUAD + (k + 1) * MMF)
                            bank = quad_im[:, k * MMF:(k + 1) * MMF]
                            nc.tensor.matmul(bank, wim_c, xt[:, hk],
                                             start=True, stop=False)
                            if act_drain:
                                nc.tensor.matmul(bank, dim_c(c), hre[:, c, hk],
                                                 start=False, stop=False)
                                nc.tensor.matmul(bank, dre_c(c), him[:, c, hk],
                                                 start=False, stop=True)
                            else:
                                nc.tensor.matmul(bank, dim_c(c), hre[:, c, hk],
                                                 start=False, stop=True)
                        if act_drain:
                            nc.scalar.copy(oim[:, c, qs], quad_im[:])
                        else:
                            nc.vector.scalar_tensor_tensor(
                                oim[:, c, qs], hims, lamsb[:, 2 + c:3 + c],
                                quad_im[:],
                                op0=mybir.AluOpType.mult, op1=mybir.AluOpType.add)

                # Bulk per-iteration stores on the ACT DGE ring (loads have
                # the SP ring to themselves). Finer-grained stores measurably
                # hurt: interleaving reads and writes at sub-iteration
                # granularity drops effective HBM bandwidth by ~15%.
                nc.scalar.dma_start(ore_v[:, :, sl], ore[:])
                nc.scalar.dma_start(oim_v[:, :, sl], oim[:])

    _split_multiwaits(nc)
    _cache["nc"] = nc
    return nc


def _split_multiwaits(nc):
    """walrus codegen allows exactly one semaphore wait per instruction.
    Move all-but-one wait of every multi-wait instruction onto single-wait
    NOP instructions spliced immediately before it on the same engine
    (engines execute their stream in order, so semantics are unchanged)."""
    k = 0
    for bb in nc.m.functions[0].blocks:
        new_list = []
        for ins in bb.instructions:
            si = ins.sync_info
            if si is not None and si.on_wait and len(si.on_wait) > 1:
                for w in si.on_wait[:-1]:
                    nop = mybir.InstNoOp(
                        name=f"WN-{k}", engine=ins.engine,
                        sync_info=mybir.SyncInfo(on_wait=[w], on_update=[]),
                    )
                    k += 1
                    new_list.append(nop)
                si.on_wait = [si.on_wait[-1]]
            new_list.append(ins)
        bb.instructions[:] = new_list


def kernel(inputs, h_re, h_im, nu_log, theta_log, B_real, B_img, gamma_log):
    global LAST_RESULTS
    inputs = np.asarray(inputs, dtype=np.float32)
    h_re = np.asarray(h_re, dtype=np.float32)
    h_im = np.asarray(h_im, dtype=np.float32)
    nu_log = np.asarray(nu_log, dtype=np.float32)
    theta_log = np.asarray(theta_log, dtype=np.float32)
    B_real = np.asarray(B_real, dtype=np.float32)
    B_img = np.asarray(B_img, dtype=np.float32)
    gamma_log = np.asarray(gamma_log, dtype=np.float32)

    # Tiny parameter math on host (matches the f32 reference computation).
    mag = np.exp(-np.exp(nu_log))          # (1, H)
    theta = np.exp(theta_log)              # (1, H)
    lam_re = (mag * np.cos(theta))[0]      # (H,)
    lam_im = (mag * np.sin(theta))[0]      # (H,)
    scale = np.exp(gamma_log).T            # (H, 1)
    w_re = (scale * B_real).T              # (IN_DIM, H)
    w_im = (scale * B_img).T               # (IN_DIM, H)

    consts = np.zeros((P, CONST_COLS), np.float32)
    consts[:, 0:HID] = w_re
    consts[:, HID:2 * HID] = w_im
    idx = np.arange(P)
    lamf = np.zeros((P, LAMF_COLS), np.float32)
    for c in range(HCHUNKS):
        lr = lam_re[c * P:(c + 1) * P]
        li = lam_im[c * P:(c + 1) * P]
        consts[idx, 2 * HID + c * P + idx] = lr
        consts[idx, 3 * HID + c * P + idx] = li
        lamf[:, c] = -li
        lamf[:, 2 + c] = lr
    consts = consts.astype(NP_BF16)

    x_bf = inputs.astype(NP_BF16)
    hre_f8 = h_re.astype(NP_FP8)
    him_f8 = h_im.astype(NP_FP8)

    in_maps = []
    for core in range(N_CORES):
        sl = slice(core * S, (core + 1) * S)
        in_maps.append({
            "x_t": np.ascontiguousarray(x_bf[sl].T),
            "hre_t": np.ascontiguousarray(hre_f8[sl].T),
            "him_t": np.ascontiguousarray(him_f8[sl].T),
            "consts": consts,
            "lamf": lamf,
        })

    nc = _build()
    res = run_bass_kernel_spmd(nc, in_maps, core_ids=list(range(N_CORES)))
    LAST_RESULTS = res

    out = np.empty((2, B_SZ, HID), np.float32)
    for core in range(N_CORES):
        sl = slice(core * S, (core + 1) * S)
        out[0, sl] = res.results[core]["o_re"].T.astype(np.float32)
        out[1, sl] = res.results[core]["o_im"].T.astype(np.float32)
    return out


# revision 22
# speedup vs baseline: 1.1815x; 1.0944x over previous
"""LRU single-step kernel for 8x TRN2 NeuronCores (Bass/Tile).

Math (per batch row b, hidden h):
  out_re[b,h] = lam_re[h]*h_re[b,h] - lam_im[h]*h_im[b,h] + (x @ (scale*B_real).T)[b,h]
  out_im[b,h] = lam_im[h]*h_re[b,h] + lam_re[h]*h_im[b,h] + (x @ (scale*B_img ).T)[b,h]

Strategy: data-parallel over the batch axis (8 shards of 32768 rows), computed
in a transposed layout (hidden on partitions, batch on the free axis). The
kernel is memory-bound, so precision is chosen per-stream to minimize HBM
bytes inside the rel-err budget:
  - x and the projection weights travel as bf16 (fp8 x fails the error gate:
    the exp(gamma) scaling amplifies x's quantization error),
  - h_re / h_im travel as fp8 e4m3 (their error is attenuated by |Lambda|<1),
  - outputs travel as bf16.

Engine split per 512-column block (PSUM bank) and hidden chunk:
  PE:   ps_re = W_re.T @ x + diag(lam_re) @ hre      (bf16 x fp8-moving)
        ps_im = W_im.T @ x + diag(lam_im) @ hre
  DVE:  ore = (him * -lam_im[p]) + ps_re             (scalar_tensor_tensor,
        oim = (him *  lam_re[p]) + ps_im              fp32 scalar, bf16 out)
The him Lambda terms ride the PSUM->SBUF drain for free, cutting PE matmuls
from 6 to 4 per block pair. Only DVE/ACT can read PSUM (GPSIMD can't, and
ACT's activation has no tensor bias), so all drains live on DVE; to fit them
under the DMA roofline each drain covers FOUR adjacent PSUM banks (2048
columns, half of PSUM) in one instruction, amortizing the fixed access
latency. The two 4-bank quad tiles ping-pong: the PE fills one while the DVE
drains the other.

DMA queues: loads on the SP ring (nothing else queued there, so prefetch never
blocks behind a store waiting on compute), stores on the ACT ring.

PE Matmult instructions only have one sync-wait slot in codegen, so waits are
carefully absorbed before real matmuls run:
  - per-iteration 1x1 "lane absorber" matmuls read one freshly-DMA'd tile each
    (and write a persistent scratch PSUM tile), so each carries exactly one
    DMA-lane wait and advances the PE's observed clock;
  - PSUM tiles are allocated once and reused manually (no pool recycling), so
    no TileRelease edges exist on PSUM: the first matmul of a group carries
    only the WAR wait on the previous iteration's PSUM->SBUF drain.
"""

import numpy as np
import ml_dtypes

import concourse.bass as bass
import concourse.mybir as mybir
from concourse.tile import TileContext
from concourse.bass_utils import run_bass_kernel_spmd

B_SZ, IN_DIM, HID = 262144, 128, 256
N_CORES = 8
S = B_SZ // N_CORES  # 32768 rows per core
P = 128
HCHUNKS = HID // P  # 2
COLS = 4096          # batch columns per outer iteration (8 KiB bf16 lines)
OUTER = S // COLS    # 8
MMF = 512            # matmul free dim (one fp32 PSUM bank)
NBLK = COLS // MMF   # 8
QUAD = 2048          # DVE drain width (four PSUM banks)

# bf16 consts layout (one (128, 1024) tensor):
#   [:, 0:256]     w_re  = (scale*B_real).T
#   [:, 256:512]   w_im  = (scale*B_img).T
#   [:, 512:768]   diag(lam_re)  chunks 0,1
#   [:, 768:1024]  diag(lam_im)  chunks 0,1
CONST_COLS = 1024
# fp32 per-partition scalars (128, 4): [-lam_im c0, -lam_im c1, lam_re c0, lam_re c1]
LAMF_COLS = 4

BF16 = mybir.dt.bfloat16
FP8 = mybir.dt.float8e4
F32 = mybir.dt.float32
NP_BF16 = ml_dtypes.bfloat16
NP_FP8 = ml_dtypes.float8_e4m3fn

_cache = {}

# Stashed BassKernelResults from the most recent run (for test harnesses).
LAST_RESULTS = None


def _build():
    if "nc" in _cache:
        return _cache["nc"]

    nc = bass.Bass(trn_type="TRN2")

    x_t = nc.dram_tensor("x_t", (IN_DIM, S), BF16, kind="ExternalInput")
    hre_t = nc.dram_tensor("hre_t", (HID, S), FP8, kind="ExternalInput")
    him_t = nc.dram_tensor("him_t", (HID, S), FP8, kind="ExternalInput")
    consts = nc.dram_tensor("consts", (P, CONST_COLS), BF16, kind="ExternalInput")
    lamf = nc.dram_tensor("lamf", (P, LAMF_COLS), F32, kind="ExternalInput")

    o_re = nc.dram_tensor("o_re", (HID, S), BF16, kind="ExternalOutput")
    o_im = nc.dram_tensor("o_im", (HID, S), BF16, kind="ExternalOutput")

    hre_v = hre_t[:, :].rearrange("(c p) s -> p c s", p=P)
    him_v = him_t[:, :].rearrange("(c p) s -> p c s", p=P)
    ore_v = o_re[:, :].rearrange("(c p) s -> p c s", p=P)
    oim_v = o_im[:, :].rearrange("(c p) s -> p c s", p=P)

    with TileContext(nc) as tc:
        with (
            tc.tile_pool(name="cpool", bufs=1) as cpool,
            tc.tile_pool(name="xin", bufs=3) as xin,
            tc.tile_pool(name="hin", bufs=3) as hin,
            tc.tile_pool(name="outp", bufs=3) as outp,
            tc.tile_pool(name="psum", bufs=1, space="PSUM") as psum,
        ):
            csb = cpool.tile([P, CONST_COLS], BF16)
            lamsb = cpool.tile([P, LAMF_COLS], F32, tag="lamf")
            nc.sync.dma_start(csb[:], consts[:, :])
            nc.sync.dma_start(lamsb[:], lamf[:, :])
            # 2 persistent 4-bank PSUM quad tiles (all 16 KiB of PSUM);
            # allocated once so no TileRelease/realloc wait sets ever form on
            # PSUM. Each quad is bank-aligned (2048 f32 = exactly 4 banks):
            # matmuls write each 512 half within a single bank while the DVE
            # drain reads the whole 2048 columns in one go.
            ps_tiles = [psum.tile([P, QUAD], F32, tag=f"ps{i}", name=f"ps{i}")
                        for i in range(2)]
            _cache["ps_idx"] = 0

            def lane_absorb(tile_ap):
                # 1x1 matmul reading the freshly-DMA'd tile: carries exactly
                # one DMA-lane wait, advancing the PE's observed clock so the
                # real matmuls don't re-wait on that lane. The write lands in
                # the corner of the quad the next real group will overwrite
                # (start=True covers it), so no scratch bank is needed.
                quad = ps_tiles[_cache["ps_idx"] % 2]
                nc.tensor.matmul(quad[0:1, 0:1], tile_ap, tile_ap,
                                 start=True, stop=True, skip_group_check=True)

            w_re_sb = csb[:, 0:HID]
            w_im_sb = csb[:, HID:2 * HID]

            def dre_c(c):
                return csb[:, 2 * HID + c * P: 2 * HID + (c + 1) * P]

            def dim_c(c):
                return csb[:, 3 * HID + c * P: 3 * HID + (c + 1) * P]

            lane_absorb(csb[0:1, 0:1])

            for o in range(OUTER):
                sl = slice(o * COLS, (o + 1) * COLS)
                xt = xin.tile([P, COLS], BF16)
                nc.sync.dma_start(xt[:], x_t[:, sl])
                hre = hin.tile([P, HCHUNKS, COLS], FP8, tag="hre")
                him = hin.tile([P, HCHUNKS, COLS], FP8, tag="him")
                nc.sync.dma_start(hre[:], hre_v[:, :, sl])
                nc.sync.dma_start(him[:], him_v[:, :, sl])
                lane_absorb(xt[0:1, 0:1])
                lane_absorb(hre[0:1, 0, 0:1])

                ore = outp.tile([P, HCHUNKS, COLS], BF16, tag="ore")
                oim = outp.tile([P, HCHUNKS, COLS], BF16, tag="oim")

                for c in range(HCHUNKS):
                    wre_c = w_re_sb[:, c * P:(c + 1) * P]
                    wim_c = w_im_sb[:, c * P:(c + 1) * P]
                    for qb in range(COLS // QUAD):
                        qs = slice(qb * QUAD, (qb + 1) * QUAD)
                        hims = him[:, c, qs]

                        quad_re = ps_tiles[_cache["ps_idx"] % 2]
                        _cache["ps_idx"] += 1
                        for k in range(QUAD // MMF):
                            hk = slice(qb * QUAD + k * MMF, qb * QUAD + (k + 1) * MMF)
                            bank = quad_re[:, k * MMF:(k + 1) * MMF]
                            nc.tensor.matmul(bank, wre_c, xt[:, hk],
                                             start=True, stop=False)
                            nc.tensor.matmul(bank, dre_c(c), hre[:, c, hk],
                                             start=False, stop=True)
                        nc.vector.scalar_tensor_tensor(
                            ore[:, c, qs], hims, lamsb[:, c:c + 1], quad_re[:],
                            op0=mybir.AluOpType.mult, op1=mybir.AluOpType.add)

                        quad_im = ps_tiles[_cache["ps_idx"] % 2]
                        _cache["ps_idx"] += 1
                        for k in range(QUAD // MMF):
                            hk = slice(qb * QUAD + k * MMF, qb * QUAD + (k + 1) * MMF)
                            bank = quad_im[:, k * MMF:(k + 1) * MMF]
                            nc.tensor.matmul(bank, wim_c, xt[:, hk],
                                             start=True, stop=False)
                            nc.tensor.matmul(bank, dim_c(c), hre[:, c, hk],
                                             start=False, stop=True)
                        nc.vector.scalar_tensor_tensor(
                            oim[:, c, qs], hims, lamsb[:, 2 + c:3 + c], quad_im[:],
                            op0=mybir.AluOpType.mult, op1=mybir.AluOpType.add)

                    if o == OUTER - 1:
                        # Last iteration only: store each chunk as soon as its
                        # quads drain, so chunk 0's flush overlaps chunk 1's
                        # compute and the final flush is half-sized. (Doing
                        # this every iteration interleaves HBM reads/writes
                        # too finely and costs ~15% effective bandwidth.)
                        nc.scalar.dma_start(ore_v[:, c:c + 1, sl],
                                            ore[:, c:c + 1, :])
                        nc.scalar.dma_start(oim_v[:, c:c + 1, sl],
                                            oim[:, c:c + 1, :])

                if o < OUTER - 1:
                    # Stores ride the ACT DGE ring; loads have the SP ring to
                    # themselves so prefetch never blocks behind these.
                    nc.scalar.dma_start(ore_v[:, :, sl], ore[:])
                    nc.scalar.dma_start(oim_v[:, :, sl], oim[:])

    _split_multiwaits(nc)
    _cache["nc"] = nc
    return nc


def _split_multiwaits(nc):
    """walrus codegen allows exactly one semaphore wait per instruction.
    Move all-but-one wait of every multi-wait instruction onto single-wait
    NOP instructions spliced immediately before it on the same engine
    (engines execute their stream in order, so semantics are unchanged)."""
    k = 0
    for bb in nc.m.functions[0].blocks:
        new_list = []
        for ins in bb.instructions:
            si = ins.sync_info
            if si is not None and si.on_wait and len(si.on_wait) > 1:
                for w in si.on_wait[:-1]:
                    nop = mybir.InstNoOp(
                        name=f"WN-{k}", engine=ins.engine,
                        sync_info=mybir.SyncInfo(on_wait=[w], on_update=[]),
                    )
                    k += 1
                    new_list.append(nop)
                si.on_wait = [si.on_wait[-1]]
            new_list.append(ins)
        bb.instructions[:] = new_list


def kernel(inputs, h_re, h_im, nu_log, theta_log, B_real, B_img, gamma_log):
    global LAST_RESULTS
    inputs = np.asarray(inputs, dtype=np.float32)
    h_re = np.asarray(h_re, dtype=np.float32)
    h_im = np.asarray(h_im, dtype=np.float32)
    nu_log = np.asarray(nu_log, dtype=np.float32)
    theta_log = np.asarray(theta_log, dtype=np.float32)
    B_real = np.asarray(B_real, dtype=np.float32)
    B_img = np.asarray(B_img, dtype=np.float32)
    gamma_log = np.asarray(gamma_log, dtype=np.float32)

    # Tiny parameter math on host (matches the f32 reference computation).
    mag = np.exp(-np.exp(nu_log))          # (1, H)
    theta = np.exp(theta_log)              # (1, H)
    lam_re = (mag * np.cos(theta))[0]      # (H,)
    lam_im = (mag * np.sin(theta))[0]      # (H,)
    scale = np.exp(gamma_log).T            # (H, 1)
    w_re = (scale * B_real).T              # (IN_DIM, H)
    w_im = (scale * B_img).T               # (IN_DIM, H)

    consts = np.zeros((P, CONST_COLS), np.float32)
    consts[:, 0:HID] = w_re
    consts[:, HID:2 * HID] = w_im
    idx = np.arange(P)
    lamf = np.zeros((P, LAMF_COLS), np.float32)
    for c in range(HCHUNKS):
        lr = lam_re[c * P:(c + 1) * P]
        li = lam_im[c * P:(c + 1) * P]
        consts[idx, 2 * HID + c * P + idx] = lr
        consts[idx, 3 * HID + c * P + idx] = li
        lamf[:, c] = -li
        lamf[:, 2 + c] = lr
    consts = consts.astype(NP_BF16)

    x_bf = inputs.astype(NP_BF16)
    hre_f8 = h_re.astype(NP_FP8)
    him_f8 = h_im.astype(NP_FP8)

    in_maps = []
    for core in range(N_CORES):
        sl = slice(core * S, (core + 1) * S)
        in_maps.append({
            "x_t": np.ascontiguousarray(x_bf[sl].T),
            "hre_t": np.ascontiguousarray(hre_f8[sl].T),
            "him_t": np.ascontiguousarray(him_f8[sl].T),
            "consts": consts,
            "lamf": lamf,
        })

    nc = _build()
    res = run_bass_kernel_spmd(nc, in_maps, core_ids=list(range(N_CORES)))
    LAST_RESULTS = res

    out = np.empty((2, B_SZ, HID), np.float32)
    for core in range(N_CORES):
        sl = slice(core * S, (core + 1) * S)
        out[0, sl] = res.results[core]["o_re"].T.astype(np.float32)
        out[1, sl] = res.results[core]["o_im"].T.astype(np.float32)
    return out


# revision 23
# speedup vs baseline: 1.1828x; 1.0011x over previous
"""LRU single-step kernel for 8x TRN2 NeuronCores (Bass/Tile).

Math (per batch row b, hidden h):
  out_re[b,h] = lam_re[h]*h_re[b,h] - lam_im[h]*h_im[b,h] + (x @ (scale*B_real).T)[b,h]
  out_im[b,h] = lam_im[h]*h_re[b,h] + lam_re[h]*h_im[b,h] + (x @ (scale*B_img ).T)[b,h]

Strategy: data-parallel over the batch axis (8 shards of 32768 rows), computed
in a transposed layout (hidden on partitions, batch on the free axis). The
kernel is memory-bound, so precision is chosen per-stream to minimize HBM
bytes inside the rel-err budget:
  - x and the projection weights travel as bf16 (fp8 x fails the error gate:
    the exp(gamma) scaling amplifies x's quantization error),
  - h_re / h_im travel as fp8 e4m3 (their error is attenuated by |Lambda|<1),
  - outputs travel as bf16.

Engine split per 512-column block (PSUM bank) and hidden chunk:
  PE:   ps_re = W_re.T @ x + diag(lam_re) @ hre      (bf16 x fp8-moving)
        ps_im = W_im.T @ x + diag(lam_im) @ hre
  DVE:  ore = (him * -lam_im[p]) + ps_re             (scalar_tensor_tensor,
        oim = (him *  lam_re[p]) + ps_im              fp32 scalar, bf16 out)
The him Lambda terms ride the PSUM->SBUF drain for free, cutting PE matmuls
from 6 to 4 per block pair. Only DVE/ACT can read PSUM (GPSIMD can't, and
ACT's activation has no tensor bias), so all drains live on DVE; to fit them
under the DMA roofline each drain covers FOUR adjacent PSUM banks (2048
columns, half of PSUM) in one instruction, amortizing the fixed access
latency. The two 4-bank quad tiles ping-pong: the PE fills one while the DVE
drains the other.

DMA queues: loads on the SP ring (nothing else queued there, so prefetch never
blocks behind a store waiting on compute), stores on the ACT ring.

PE Matmult instructions only have one sync-wait slot in codegen, so waits are
carefully absorbed before real matmuls run:
  - per-iteration 1x1 "lane absorber" matmuls read one freshly-DMA'd tile each
    (and write a persistent scratch PSUM tile), so each carries exactly one
    DMA-lane wait and advances the PE's observed clock;
  - PSUM tiles are allocated once and reused manually (no pool recycling), so
    no TileRelease edges exist on PSUM: the first matmul of a group carries
    only the WAR wait on the previous iteration's PSUM->SBUF drain.
"""

import numpy as np
import ml_dtypes

import concourse.bass as bass
import concourse.mybir as mybir
from concourse.tile import TileContext
from concourse.bass_utils import run_bass_kernel_spmd

B_SZ, IN_DIM, HID = 262144, 128, 256
N_CORES = 8
S = B_SZ // N_CORES  # 32768 rows per core
P = 128
HCHUNKS = HID // P  # 2
COLS = 4096          # batch columns per outer iteration (8 KiB bf16 lines)
OUTER = S // COLS    # 8
MMF = 512            # matmul free dim (one fp32 PSUM bank)
NBLK = COLS // MMF   # 8
QUAD = 2048          # DVE drain width (four PSUM banks)

# bf16 consts layout (one (128, 1024) tensor):
#   [:, 0:256]     w_re  = (scale*B_real).T
#   [:, 256:512]   w_im  = (scale*B_img).T
#   [:, 512:768]   diag(lam_re)  chunks 0,1
#   [:, 768:1024]  diag(lam_im)  chunks 0,1
CONST_COLS = 1024
# fp32 per-partition scalars (128, 4): [-lam_im c0, -lam_im c1, lam_re c0, lam_re c1]
LAMF_COLS = 4

BF16 = mybir.dt.bfloat16
FP8 = mybir.dt.float8e4
F32 = mybir.dt.float32
NP_BF16 = ml_dtypes.bfloat16
NP_FP8 = ml_dtypes.float8_e4m3fn

_cache = {}

# Stashed BassKernelResults from the most recent run (for test harnesses).
LAST_RESULTS = None


def _build():
    if "nc" in _cache:
        return _cache["nc"]

    nc = bass.Bass(trn_type="TRN2")

    x_t = nc.dram_tensor("x_t", (IN_DIM, S), BF16, kind="ExternalInput")
    hre_t = nc.dram_tensor("hre_t", (HID, S), FP8, kind="ExternalInput")
    him_t = nc.dram_tensor("him_t", (HID, S), FP8, kind="ExternalInput")
    consts = nc.dram_tensor("consts", (P, CONST_COLS), BF16, kind="ExternalInput")
    lamf = nc.dram_tensor("lamf", (P, LAMF_COLS), F32, kind="ExternalInput")

    o_re = nc.dram_tensor("o_re", (HID, S), BF16, kind="ExternalOutput")
    o_im = nc.dram_tensor("o_im", (HID, S), BF16, kind="ExternalOutput")

    hre_v = hre_t[:, :].rearrange("(c p) s -> p c s", p=P)
    him_v = him_t[:, :].rearrange("(c p) s -> p c s", p=P)
    ore_v = o_re[:, :].rearrange("(c p) s -> p c s", p=P)
    oim_v = o_im[:, :].rearrange("(c p) s -> p c s", p=P)

    with TileContext(nc) as tc:
        with (
            tc.tile_pool(name="cpool", bufs=1) as cpool,
            tc.tile_pool(name="xin", bufs=4) as xin,
            tc.tile_pool(name="hin", bufs=4) as hin,
            tc.tile_pool(name="outp", bufs=3) as outp,
            tc.tile_pool(name="psum", bufs=1, space="PSUM") as psum,
        ):
            csb = cpool.tile([P, CONST_COLS], BF16)
            lamsb = cpool.tile([P, LAMF_COLS], F32, tag="lamf")
            nc.sync.dma_start(csb[:], consts[:, :])
            nc.sync.dma_start(lamsb[:], lamf[:, :])
            # 2 persistent 4-bank PSUM quad tiles (all 16 KiB of PSUM);
            # allocated once so no TileRelease/realloc wait sets ever form on
            # PSUM. Each quad is bank-aligned (2048 f32 = exactly 4 banks):
            # matmuls write each 512 half within a single bank while the DVE
            # drain reads the whole 2048 columns in one go.
            ps_tiles = [psum.tile([P, QUAD], F32, tag=f"ps{i}", name=f"ps{i}")
                        for i in range(2)]
            _cache["ps_idx"] = 0

            def lane_absorb(tile_ap):
                # 1x1 matmul reading the freshly-DMA'd tile: carries exactly
                # one DMA-lane wait, advancing the PE's observed clock so the
                # real matmuls don't re-wait on that lane. The write lands in
                # the corner of the quad the next real group will overwrite
                # (start=True covers it), so no scratch bank is needed.
                quad = ps_tiles[_cache["ps_idx"] % 2]
                nc.tensor.matmul(quad[0:1, 0:1], tile_ap, tile_ap,
                                 start=True, stop=True, skip_group_check=True)

            w_re_sb = csb[:, 0:HID]
            w_im_sb = csb[:, HID:2 * HID]

            def dre_c(c):
                return csb[:, 2 * HID + c * P: 2 * HID + (c + 1) * P]

            def dim_c(c):
                return csb[:, 3 * HID + c * P: 3 * HID + (c + 1) * P]

            lane_absorb(csb[0:1, 0:1])

            for o in range(OUTER):
                sl = slice(o * COLS, (o + 1) * COLS)
                xt = xin.tile([P, COLS], BF16)
                nc.sync.dma_start(xt[:], x_t[:, sl])
                hre = hin.tile([P, HCHUNKS, COLS], FP8, tag="hre")
                him = hin.tile([P, HCHUNKS, COLS], FP8, tag="him")
                nc.sync.dma_start(hre[:], hre_v[:, :, sl])
                nc.sync.dma_start(him[:], him_v[:, :, sl])
                lane_absorb(xt[0:1, 0:1])
                lane_absorb(hre[0:1, 0, 0:1])

                ore = outp.tile([P, HCHUNKS, COLS], BF16, tag="ore")
                oim = outp.tile([P, HCHUNKS, COLS], BF16, tag="oim")

                for c in range(HCHUNKS):
                    wre_c = w_re_sb[:, c * P:(c + 1) * P]
                    wim_c = w_im_sb[:, c * P:(c + 1) * P]
                    for qb in range(COLS // QUAD):
                        qs = slice(qb * QUAD, (qb + 1) * QUAD)
                        hims = him[:, c, qs]

                        quad_re = ps_tiles[_cache["ps_idx"] % 2]
                        _cache["ps_idx"] += 1
                        for k in range(QUAD // MMF):
                            hk = slice(qb * QUAD + k * MMF, qb * QUAD + (k + 1) * MMF)
                            bank = quad_re[:, k * MMF:(k + 1) * MMF]
                            nc.tensor.matmul(bank, wre_c, xt[:, hk],
                                             start=True, stop=False)
                            nc.tensor.matmul(bank, dre_c(c), hre[:, c, hk],
                                             start=False, stop=True)
                        nc.vector.scalar_tensor_tensor(
                            ore[:, c, qs], hims, lamsb[:, c:c + 1], quad_re[:],
                            op0=mybir.AluOpType.mult, op1=mybir.AluOpType.add)

                        quad_im = ps_tiles[_cache["ps_idx"] % 2]
                        _cache["ps_idx"] += 1
                        for k in range(QUAD // MMF):
                            hk = slice(qb * QUAD + k * MMF, qb * QUAD + (k + 1) * MMF)
                            bank = quad_im[:, k * MMF:(k + 1) * MMF]
                            nc.tensor.matmul(bank, wim_c, xt[:, hk],
                                             start=True, stop=False)
                            nc.tensor.matmul(bank, dim_c(c), hre[:, c, hk],
                                             start=False, stop=True)
                        nc.vector.scalar_tensor_tensor(
                            oim[:, c, qs], hims, lamsb[:, 2 + c:3 + c], quad_im[:],
                            op0=mybir.AluOpType.mult, op1=mybir.AluOpType.add)

                    if o == OUTER - 1:
                        # Last iteration only: store each chunk as soon as its
                        # quads drain, so chunk 0's flush overlaps chunk 1's
                        # compute and the final flush is half-sized. (Doing
                        # this every iteration interleaves HBM reads/writes
                        # too finely and costs ~15% effective bandwidth.)
                        nc.scalar.dma_start(ore_v[:, c:c + 1, sl],
                                            ore[:, c:c + 1, :])
                        nc.scalar.dma_start(oim_v[:, c:c + 1, sl],
                                            oim[:, c:c + 1, :])

                if o < OUTER - 1:
                    # Stores ride the ACT DGE ring; loads have the SP ring to
                    # themselves so prefetch never blocks behind these.
                    nc.scalar.dma_start(ore_v[:, :, sl], ore[:])
                    nc.scalar.dma_start(oim_v[:, :, sl], oim[:])

    _split_multiwaits(nc)
    _cache["nc"] = nc
    return nc


def _split_multiwaits(nc):
    """walrus codegen allows exactly one semaphore wait per instruction.
    Move all-but-one wait of every multi-wait instruction onto single-wait
    NOP instructions spliced immediately before it on the same engine
    (engines execute their stream in order, so semantics are unchanged)."""
    k = 0
    for bb in nc.m.functions[0].blocks:
        new_list = []
        for ins in bb.instructions:
            si = ins.sync_info
            if si is not None and si.on_wait and len(si.on_wait) > 1:
                for w in si.on_wait[:-1]:
                    nop = mybir.InstNoOp(
                        name=f"WN-{k}", engine=ins.engine,
                        sync_info=mybir.SyncInfo(on_wait=[w], on_update=[]),
                    )
                    k += 1
                    new_list.append(nop)
                si.on_wait = [si.on_wait[-1]]
            new_list.append(ins)
        bb.instructions[:] = new_list


def kernel(inputs, h_re, h_im, nu_log, theta_log, B_real, B_img, gamma_log):
    global LAST_RESULTS
    inputs = np.asarray(inputs, dtype=np.float32)
    h_re = np.asarray(h_re, dtype=np.float32)
    h_im = np.asarray(h_im, dtype=np.float32)
    nu_log = np.asarray(nu_log, dtype=np.float32)
    theta_log = np.asarray(theta_log, dtype=np.float32)
    B_real = np.asarray(B_real, dtype=np.float32)
    B_img = np.asarray(B_img, dtype=np.float32)
    gamma_log = np.asarray(gamma_log, dtype=np.float32)

    # Tiny parameter math on host (matches the f32 reference computation).
    mag = np.exp(-np.exp(nu_log))          # (1, H)
    theta = np.exp(theta_log)              # (1, H)
    lam_re = (mag * np.cos(theta))[0]      # (H,)
    lam_im = (mag * np.sin(theta))[0]      # (H,)
    scale = np.exp(gamma_log).T            # (H, 1)
    w_re = (scale * B_real).T              # (IN_DIM, H)
    w_im = (scale * B_img).T               # (IN_DIM, H)

    consts = np.zeros((P, CONST_COLS), np.float32)
    consts[:, 0:HID] = w_re
    consts[:, HID:2 * HID] = w_im
    idx = np.arange(P)
    lamf = np.zeros((P, LAMF_COLS), np.float32)
    for c in range(HCHUNKS):
        lr = lam_re[c * P:(c + 1) * P]
        li = lam_im[c * P:(c + 1) * P]
        consts[idx, 2 * HID + c * P + idx] = lr
        consts[idx, 3 * HID + c * P + idx] = li
        lamf[:, c] = -li
        lamf[:, 2 + c] = lr
    consts = consts.astype(NP_BF16)

    x_bf = inputs.astype(NP_BF16)
    hre_f8 = h_re.astype(NP_FP8)
    him_f8 = h_im.astype(NP_FP8)

    in_maps = []
    for core in range(N_CORES):
        sl = slice(core * S, (core + 1) * S)
        in_maps.append({
            "x_t": np.ascontiguousarray(x_bf[sl].T),
            "hre_t": np.ascontiguousarray(hre_f8[sl].T),
            "him_t": np.ascontiguousarray(him_f8[sl].T),
            "consts": consts,
            "lamf": lamf,
        })

    nc = _build()
    res = run_bass_kernel_spmd(nc, in_maps, core_ids=list(range(N_CORES)))
    LAST_RESULTS = res

    out = np.empty((2, B_SZ, HID), np.float32)
    for core in range(N_CORES):
        sl = slice(core * S, (core + 1) * S)
        out[0, sl] = res.results[core]["o_re"].T.astype(np.float32)
        out[1, sl] = res.results[core]["o_im"].T.astype(np.float32)
    return out
